# revision 59
# baseline (speedup 1.0000x reference)
"""BiMamba block (bidirectional Mamba-1 + layernorm) as a Bass/Tile kernel
for 8 Trainium2 NeuronCores.

Sharding: data-parallel over batch — core i computes batch row i end-to-end
(both scan directions + layernorm), no collectives.

Per-core layout: channel-major [channel(partition), time(free)] activations
until the output projection, which emits token-major [token, d_model].

The depthwise conv and the D-skip connection run on the PE as diag(w)/diag(D)
stationary matmuls (the D-skip opens the same PSUM accumulation group that the
per-state ident matmuls accumulate into), freeing DVE cycles for the scans.

Selective scan per (d-block of 128 channels, state index n) on [128, L] tiles:
    a = exp(A[:,n] * dt)            ACT, per-partition scale AP
    b = (dt*u) * B_bcast[n]         DVE tensor_tensor bf16 (2x mode)
    h = tensor_tensor_scan(a, b)    DVE, fp32 internal recurrence state
    q = h * C_bcast[n]              DVE tensor_tensor bf16 (2x mode)
    psum_y += I @ q                 PE accumulates the sum over n in PSUM

B/C broadcasts are DMA re-reads of a small DRAM staging row with a
partition-step-0 access pattern.  The backward direction runs on
host-reversed input; un-reversal is free via a negative-stride output AP at
the yf write.
"""

import os
import sys
from contextlib import ExitStack

for _p in ("/opt/trn_rl_repo", "/root/.axon_site/_ro/trn_rl_repo"):
    if os.path.isdir(_p) and _p not in sys.path:
        sys.path.insert(0, _p)

import numpy as np
import ml_dtypes

import concourse.bass as bass
import concourse.tile as tile
from concourse import bacc, mybir
from concourse.masks import make_identity

AF = mybir.ActivationFunctionType
ALU = mybir.AluOpType
F32 = mybir.dt.float32
F32R = mybir.dt.float32r
BF16 = mybir.dt.bfloat16
F16 = mybir.dt.float16

D_MODEL = 512
D_STATE = 16
D_CONV = 4
D_INNER = 1024
DT_RANK = 32
NB = D_INNER // 128          # 8 d-blocks
KM = D_MODEL // 128          # 4 k-tiles over d_model
LN_EPS = 1e-5

XZ_F32R = False              # xz matmul in float32r (else bf16)
SCAN_B_BF16 = True           # scan data1 dtype bf16 (else fp32)
STOP_AFTER = os.environ.get("BIMAMBA_STOP_AFTER", "")


def host_prep(inputs: dict, l_override: int | None = None) -> tuple[list[dict], int]:
    """Full problem inputs -> per-core in_maps (one batch row per core)."""
    x = np.asarray(inputs["x"], dtype=np.float32)
    Bsz, L, _ = x.shape
    if l_override is not None:
        L = l_override
        x = x[:, :L]
    bf = ml_dtypes.bfloat16

    def pack(a, nblk):  # [nblk*128, F] -> [128, nblk*F]
        return np.concatenate([a[i * 128:(i + 1) * 128] for i in range(nblk)], axis=1).copy()

    shared = {}
    for p in ("f", "b"):
        Wxz = np.asarray(inputs[f"{p}_Wxz"], np.float32)
        shared[f"{p}_Wxz"] = pack(Wxz, KM) if XZ_F32R else pack(Wxz, KM).astype(bf)
        cw = np.asarray(inputs[f"{p}_conv_w"], np.float32).reshape(D_INNER, D_CONV)
        shared[f"{p}_convw"] = pack(cw, NB)
        cb = np.asarray(inputs[f"{p}_conv_b"], np.float32).reshape(D_INNER, 1)
        shared[f"{p}_convb"] = pack(cb, NB)
        Wxm = np.asarray(inputs[f"{p}_Wx"], np.float32)
        Wxp = np.zeros((D_INNER, 80), np.float32)
        Wxp[:, 0:48] = Wxm[:, 0:48]
        Wxp[:, 64:80] = Wxm[:, 48:64]
        shared[f"{p}_Wx"] = pack(Wxp, NB).astype(bf)
        Wdtf = np.asarray(inputs[f"{p}_Wdt"], np.float32)
        Wdt_hi = Wdtf.astype(bf)
        Wdt_lo = (Wdtf - Wdt_hi.astype(np.float32)).astype(bf)
        shared[f"{p}_Wdt"] = np.concatenate([Wdt_hi, Wdt_lo], axis=1).copy()
        bdt = np.asarray(inputs[f"{p}_bdt"], np.float32).reshape(D_INNER, 1)
        shared[f"{p}_bdt"] = pack(bdt, NB)
        # positive-dt convention: dtt = softplus(...), exp scale = A (negative)
        negA = -np.exp(np.asarray(inputs[f"{p}_A_log"], np.float32))
        shared[f"{p}_A"] = pack(negA, NB)
        Dv = np.asarray(inputs[f"{p}_D"], np.float32).reshape(D_INNER, 1)
        shared[f"{p}_D"] = pack(Dv, NB)
        shared[f"{p}_Wout"] = pack(np.asarray(inputs[f"{p}_Wout"], np.float32), NB).astype(bf)
    shared["ln_g"] = np.broadcast_to(np.asarray(inputs["ln_g"], np.float32)[None, :], (128, D_MODEL)).copy()
    shared["ln_b"] = np.broadcast_to(np.asarray(inputs["ln_b"], np.float32)[None, :], (128, D_MODEL)).copy()

    in_maps = []
    for bi in range(Bsz):
        xT = np.ascontiguousarray(x[bi].T)        # [512, L]
        xTr = np.ascontiguousarray(x[bi][::-1].T)
        m = dict(shared)
        m["xT"] = pack(xT, KM) if XZ_F32R else pack(xT, KM).astype(bf)
        m["xTr"] = pack(xTr, KM) if XZ_F32R else pack(xTr, KM).astype(bf)
        in_maps.append(m)
    return in_maps, L


def declare_ios(nc: bass.Bass, L: int) -> dict:
    io = {}
    xdt = F32R if XZ_F32R else BF16
    io["xT"] = nc.dram_tensor("xT", [128, KM * L], xdt, kind="ExternalInput").ap()
    io["xTr"] = nc.dram_tensor("xTr", [128, KM * L], xdt, kind="ExternalInput").ap()
    for p in ("f", "b"):
        io[f"{p}_Wxz"] = nc.dram_tensor(f"{p}_Wxz", [128, KM * 2 * D_INNER], xdt, kind="ExternalInput").ap()
        io[f"{p}_convw"] = nc.dram_tensor(f"{p}_convw", [128, NB * D_CONV], F32, kind="ExternalInput").ap()
        io[f"{p}_convb"] = nc.dram_tensor(f"{p}_convb", [128, NB], F32, kind="ExternalInput").ap()
        io[f"{p}_Wx"] = nc.dram_tensor(f"{p}_Wx", [128, NB * 80], BF16, kind="ExternalInput").ap()
        io[f"{p}_Wdt"] = nc.dram_tensor(f"{p}_Wdt", [32, 2 * D_INNER], BF16, kind="ExternalInput").ap()
        io[f"{p}_bdt"] = nc.dram_tensor(f"{p}_bdt", [128, NB], F32, kind="ExternalInput").ap()
        io[f"{p}_A"] = nc.dram_tensor(f"{p}_A", [128, NB * D_STATE], F32, kind="ExternalInput").ap()
        io[f"{p}_D"] = nc.dram_tensor(f"{p}_D", [128, NB], F32, kind="ExternalInput").ap()
        io[f"{p}_Wout"] = nc.dram_tensor(f"{p}_Wout", [128, NB * D_MODEL], BF16, kind="ExternalInput").ap()
    io["ln_g"] = nc.dram_tensor("ln_g", [128, D_MODEL], F32, kind="ExternalInput").ap()
    io["ln_b"] = nc.dram_tensor("ln_b", [128, D_MODEL], F32, kind="ExternalInput").ap()
    io["out"] = nc.dram_tensor("out", [L, D_MODEL], F32, kind="ExternalOutput").ap()
    return io


def build_kernel(ctx: ExitStack, tc: tile.TileContext, io: dict, L: int):
    nc = tc.nc
    FC = min(512, L)
    FT = L // FC                 # 512-wide free chunks
    MT = L // 128                # token tiles
    HM = D_INNER // 128          # m-tiles per xz half (8)

    wpool = ctx.enter_context(tc.tile_pool(name="wglob", bufs=1))
    ident = wpool.tile([128, 128], BF16, tag="ident")
    make_identity(nc, ident[:])
    ln_g = wpool.tile([128, D_MODEL], F32, tag="ln_g")
    nc.sync.dma_start(ln_g[:], io["ln_g"])
    ln_b = wpool.tile([128, D_MODEL], F32, tag="ln_b")
    nc.sync.dma_start(ln_b[:], io["ln_b"])
    dglob = ctx.enter_context(tc.tile_pool(name="dglob", bufs=1, space="DRAM"))
    s_d = dglob.tile([128, MT * D_MODEL], F32, tag="s_d")

    def setup_dir(p, dctx):
        S = {"p": p, "dctx": dctx}
        aw = dctx.enter_context(tc.tile_pool(name=f"aw{p}", bufs=1))
        S["aw"] = aw
        dpool = dctx.enter_context(tc.tile_pool(name=f"dram{p}", bufs=1, space="DRAM"))
        S["zs_d"] = dpool.tile([128, NB * L], BF16, tag="zs_d", name=f"zs_d_{p}")
        S["bc_d"] = dpool.tile([32, L], BF16, tag="bc_d", name=f"bc_d_{p}")
        S["uc_d"] = dpool.tile([128, NB * L], BF16, tag="uc_d", name=f"uc_d_{p}")
        S["yf_d"] = dpool.tile([128, NB * L], BF16, tag="yf_d", name=f"yf_d_{p}")
        S["wx"] = aw.tile([128, NB * 80], BF16, tag="wx", name=f"wx_{p}")
        nc.sync.dma_start(S["wx"][:], io[f"{p}_Wx"])
        S["amat"] = aw.tile([128, NB * D_STATE], F32, tag="amat", name=f"amat_{p}")
        nc.sync.dma_start(S["amat"][:], io[f"{p}_A"])
        dmat = aw.tile([128, NB], F32, tag="dmat")
        nc.sync.dma_start(dmat[:], io[f"{p}_D"])
        convw_g = aw.tile([128, NB * D_CONV], F32, tag="convw_g", name=f"convw_g_{p}")
        nc.sync.dma_start(convw_g[:], io[f"{p}_convw"])
        S["convw_g"] = convw_g
        S["convb"] = aw.tile([128, NB], F32, tag="convb", name=f"convb_{p}")
        nc.sync.dma_start(S["convb"][:], io[f"{p}_convb"])
        diagd = [aw.tile([128, 128], BF16, tag=f"dgd{d}", name=f"dgd{d}_{p}")
                 for d in range(NB)]
        for d in range(NB):
            nc.vector.tensor_scalar_mul(diagd[d][:], ident[:], dmat[:, d:d + 1])
        S["diagd"] = diagd
        return S

    def emit_A_steps(S, lean):
        """Phase A: xz matmul, conv on PE, silu.

        lean=True (overlapped direction): small buffers, uc staged to DRAM.
        lean=False: full buffers, silu written straight into SBUF uc tiles.
        Generator yielding after each (half, m8) unit (16 yields)."""
        p = S["p"]
        with ExitStack() as actx:
            apool = actx.enter_context(tc.tile_pool(name=f"pa{p}", bufs=1))
            whpool = actx.enter_context(tc.tile_pool(name=f"wh{p}", bufs=1 if lean else 2))
            u0pool = actx.enter_context(tc.tile_pool(name=f"u0p{p}", bufs=2 if lean else 3))
            dwpool = actx.enter_context(tc.tile_pool(name=f"dw{p}", bufs=2))
            evpool = actx.enter_context(tc.tile_pool(name=f"ev{p}", bufs=2 if lean else 3))
            psA = actx.enter_context(tc.tile_pool(name=f"psA{p}", bufs=1 if lean else 2, space="PSUM"))

            xin = apool.tile([128, KM * L], BF16, tag="xin")
            xsrc = io["xT" if p == "f" else "xTr"]
            for k in range(KM):
                nc.sync.dma_start(xin[:, k * L:(k + 1) * L], xsrc[:, k * L:(k + 1) * L])
            for half in range(2):        # 0: u-channels, 1: z-channels
                wh = whpool.tile([128, KM * D_INNER], BF16, tag="wh")
                wsrc = io[f"{p}_Wxz"].rearrange("p (k c) -> p k c", k=KM)[
                    :, :, half * D_INNER:(half + 1) * D_INNER]
                for k in range(KM):
                    nc.sync.dma_start(wh[:, k * D_INNER:(k + 1) * D_INNER], wsrc[:, k, :])
                for m8 in range(HM):
                    u0t = None
                    dgw = None
                    if half == 0:
                        u0t = u0pool.tile([128, D_CONV - 1 + L], BF16, tag="u0")
                        nc.gpsimd.memset(u0t[:, 0:D_CONV - 1], 0.0)
                        dgw = [dwpool.tile([128, 128], BF16, tag=f"dgw{j}",
                                           name=f"dgw{p}_{m8}_{j}") for j in range(D_CONV)]
                        for j in range(D_CONV):
                            nc.vector.tensor_scalar_mul(
                                dgw[j][:], ident[:],
                                S["convw_g"][:, m8 * D_CONV + j: m8 * D_CONV + j + 1])
                    for fp in range(FT // 2):
                        # k-outer over an f-pair: each stationary k-slice
                        # serves 2 matmuls back-to-back (same-stationary
                        # matmuls are ~2x cheaper on HW)
                        pss = [psA.tile([128, FC], F32, tag=f"pxz{i}",
                                        name=f"pxz{p}_{half}_{m8}_{fp}_{i}")
                               for i in range(2)]
                        for k in range(KM):
                            for i in range(2):
                                f = 2 * fp + i
                                nc.tensor.matmul(
                                    pss[i][:],
                                    wh[:, k * D_INNER + m8 * 128: k * D_INNER + (m8 + 1) * 128],
                                    xin[:, k * L + f * FC: k * L + (f + 1) * FC],
                                    start=(k == 0), stop=(k == KM - 1),
                                )
                        for i in range(2):
                            f = 2 * fp + i
                            if half == 0:
                                nc.scalar.copy(
                                    u0t[:, D_CONV - 1 + f * FC: D_CONV - 1 + (f + 1) * FC],
                                    pss[i][:])
                            else:
                                zt = evpool.tile([128, FC], BF16, tag="zt")
                                if lean:
                                    nc.scalar.copy(zt[:], pss[i][:])   # raw z; silu later
                                else:
                                    nc.scalar.activation(zt[:], pss[i][:], AF.Silu)
                                nc.sync.dma_start(
                                    S["zs_d"][:, m8 * L + f * FC: m8 * L + (f + 1) * FC], zt[:])
                    if half == 0:
                        # depthwise causal conv via diag(w_j) matmuls,
                        # j-outer over f-pairs for stationary reuse
                        for fp in range(FT // 2):
                            pcs = [psA.tile([128, FC], F32, tag=f"pcv{i}",
                                            name=f"pcv{p}_{m8}_{fp}_{i}")
                                   for i in range(2)]
                            for j in range(D_CONV):
                                for i in range(2):
                                    f = 2 * fp + i
                                    nc.tensor.matmul(
                                        pcs[i][:], dgw[j][:],
                                        u0t[:, f * FC + j: f * FC + j + FC],
                                        start=(j == 0), stop=(j == D_CONV - 1))
                            for i in range(2):
                                f = 2 * fp + i
                                if lean:
                                    # identity+bias copy; silu deferred to load_uc
                                    ut = evpool.tile([128, FC], BF16, tag="ut")
                                    nc.scalar.activation(ut[:], pcs[i][:], AF.Identity,
                                                         bias=S["convb"][:, m8:m8 + 1])
                                    nc.sync.dma_start(
                                        S["uc_d"][:, m8 * L + f * FC: m8 * L + (f + 1) * FC], ut[:])
                                else:
                                    nc.scalar.activation(
                                        S["uc"][m8][:, f * FC:(f + 1) * FC], pcs[i][:],
                                        AF.Silu, bias=S["convb"][:, m8:m8 + 1])
                    yield

    def alloc_uc(S):
        p = S["p"]
        S["ucdt_ctx"] = S["dctx"].enter_context(ExitStack())
        ucpool = S["ucdt_ctx"].enter_context(tc.tile_pool(name=f"ucp{p}", bufs=1))
        S["uc"] = [ucpool.tile([128, L], BF16, tag=f"uc{d}", name=f"uc{d}_{p}")
                   for d in range(NB)]

    def load_uc(S):
        for d in range(NB):
            nc.sync.dma_start(S["uc"][d][:], S["uc_d"][:, d * L:(d + 1) * L])
        zpool = S["ucdt_ctx"].enter_context(tc.tile_pool(name=f"zsil{S['p']}", bufs=2))
        for d in range(NB):
            nc.scalar.activation(S["uc"][d][:], S["uc"][d][:], AF.Silu)
            zr = zpool.tile([128, L], BF16, tag="zr")
            nc.sync.dma_start(zr[:], S["zs_d"][:, d * L:(d + 1) * L])
            nc.scalar.activation(zr[:], zr[:], AF.Silu)
            nc.sync.dma_start(S["zs_d"][:, d * L:(d + 1) * L], zr[:])

    def emit_B(S):
        p = S["p"]
        uc = S["uc"]
        dtpool = S["ucdt_ctx"].enter_context(tc.tile_pool(name=f"dtp{p}", bufs=1))
        S["dtt"] = [dtpool.tile([128, L], F16, tag=f"dt{d}", name=f"dt{d}_{p}")
                    for d in range(NB)]
        dtt = S["dtt"]
        with ExitStack() as bctx:
            bpool = bctx.enter_context(tc.tile_pool(name=f"pb{p}", bufs=1))
            psB = bctx.enter_context(tc.tile_pool(name=f"psB{p}", bufs=1, space="PSUM"))
            wdt = bpool.tile([32, 2 * D_INNER], BF16, tag="wdt")
            nc.sync.dma_start(wdt[:], io[f"{p}_Wdt"])
            bdt = bpool.tile([128, NB], F32, tag="bdt")
            nc.sync.dma_start(bdt[:], io[f"{p}_bdt"])
            dtr = bpool.tile([32, L], F32, tag="dtr")
            dtr_hi = bpool.tile([32, L], BF16, tag="dtr_hi")
            dtr_lo = bpool.tile([32, L], BF16, tag="dtr_lo")
            bmr = bpool.tile([16, L], BF16, tag="bmr")
            cmr = bpool.tile([16, L], BF16, tag="cmr")
            pxs = [psB.tile([80, FC], F32, tag=f"pxd{f}", name=f"pxd{p}_{f}", bufs=1)
                   for f in range(FT)]
            for k in range(NB):
                for f in range(FT):
                    nc.tensor.matmul(pxs[f][:], S["wx"][:, k * 80:(k + 1) * 80],
                                     uc[k][:, f * FC:(f + 1) * FC],
                                     start=(k == 0), stop=(k == NB - 1))
            for f in range(FT):
                ps = pxs[f]
                nc.scalar.copy(dtr[:, f * FC:(f + 1) * FC], ps[0:DT_RANK, :])
                nc.scalar.copy(dtr_hi[:, f * FC:(f + 1) * FC], ps[0:DT_RANK, :])
                nc.scalar.copy(bmr[:, f * FC:(f + 1) * FC], ps[DT_RANK:DT_RANK + D_STATE, :])
                nc.scalar.copy(cmr[:, f * FC:(f + 1) * FC], ps[64:80, :])
            nc.sync.dma_start(S["bc_d"][0:16, :], bmr[:])
            nc.sync.dma_start(S["bc_d"][16:32, :], cmr[:])
            nc.vector.tensor_tensor(out=dtr_lo[:], in0=dtr[:], in1=dtr_hi[:],
                                    op=ALU.subtract)
            # dtt = softplus(dtproj + bdt) = ln(exp(dtproj + bdt) + 1).
            # exp chunks all batched first, then the lns: the compiler maps
            # exp and ln to different act tables, so interleaving them costs
            # a 1283ns table load per switch.
            sg = [bpool.tile([128, L], F32, tag=f"sg{d}", name=f"sg{d}_{p}")
                  for d in range(NB)]
            for d in range(NB):
                # stationary-grouped: the hi slice serves 8 matmuls (4 chunks
                # x 2 movings) back-to-back, then the lo slice serves 4
                pds = [psB.tile([128, FC], F32, tag=f"pdt{f}",
                                name=f"pdt{p}_{d}_{f}") for f in range(FT)]
                for f in range(FT):
                    nc.tensor.matmul(pds[f][:], wdt[:, d * 128:(d + 1) * 128],
                                     dtr_hi[:, f * FC:(f + 1) * FC],
                                     start=True, stop=False)
                for f in range(FT):
                    nc.tensor.matmul(pds[f][:], wdt[:, d * 128:(d + 1) * 128],
                                     dtr_lo[:, f * FC:(f + 1) * FC],
                                     start=False, stop=False)
                for f in range(FT):
                    nc.tensor.matmul(pds[f][:], wdt[:, D_INNER + d * 128: D_INNER + (d + 1) * 128],
                                     dtr_hi[:, f * FC:(f + 1) * FC],
                                     start=False, stop=True)
                for f in range(FT):
                    nc.scalar.activation(sg[d][:, f * FC:(f + 1) * FC], pds[f][:],
                                         AF.Exp, bias=bdt[:, d:d + 1])
            for d in range(NB):
                nc.scalar.activation(dtt[d][:], sg[d][:], AF.Ln, bias=1.0)

    def setup_S(S):
        p = S["p"]
        sctx = ExitStack()
        S["sctx"] = sctx
        S["scanpool"] = sctx.enter_context(tc.tile_pool(name=f"sc{p}", bufs=2))
        S["qpool"] = sctx.enter_context(tc.tile_pool(name=f"q{p}", bufs=1))
        S["bcpool"] = sctx.enter_context(tc.tile_pool(name=f"bc{p}", bufs=2))
        S["psY"] = sctx.enter_context(tc.tile_pool(name=f"psY{p}", bufs=1, space="PSUM"))

    def emit_S_d(S, d):
        p = S["p"]
        scanpool, qpool, bcpool, psY = S["scanpool"], S["qpool"], S["bcpool"], S["psY"]
        uc, dtt, bc_d, zs_d, amat, diagd = (S["uc"], S["dtt"], S["bc_d"],
                                            S["zs_d"], S["amat"], S["diagd"])
        dtu = scanpool.tile([128, L], BF16, tag="dtu", bufs=1)
        nc.vector.tensor_tensor(out=dtu[:], in0=dtt[d][:], in1=uc[d][:], op=ALU.mult)
        zst = scanpool.tile([128, L], BF16, tag="zst", bufs=1)
        nc.sync.dma_start(zst[:], zs_d[:, d * L:(d + 1) * L])
        py = psY.tile([128, L], F32, tag="py")
        for f in range(FT):
            nc.tensor.matmul(py[:, f * FC:(f + 1) * FC], diagd[d][:],
                             uc[d][:, f * FC:(f + 1) * FC],
                             start=True, stop=False)
        for np2 in range(D_STATE // 2):
            n0 = 2 * np2
            cb2 = bcpool.tile([128, 2, L], BF16, tag="cb", bufs=1)
            nc.sync.dma_start(cb2[:, 0, :], bc_d[16 + n0:17 + n0, :].broadcast_to((128, L)))
            nc.sync.dma_start(cb2[:, 1, :], bc_d[17 + n0:18 + n0, :].broadcast_to((128, L)))
            h2 = scanpool.tile([128, 2, L], BF16, tag="h")
            for i in (0, 1):
                n = n0 + i
                a = scanpool.tile([128, L], F16, tag="a")
                nc.scalar.activation(a[:], dtt[d][:], AF.Exp,
                                     scale=amat[:, d * D_STATE + n: d * D_STATE + n + 1])
                bb = bcpool.tile([128, L], BF16, tag="bb")
                nc.sync.dma_start(bb[:], bc_d[n:n + 1, :].broadcast_to((128, L)))
                bt = scanpool.tile([128, L], BF16, tag="bt")
                nc.vector.tensor_tensor(out=bt[:], in0=dtu[:], in1=bb[:], op=ALU.mult)
                nc.vector.tensor_tensor_scan(h2[:, i, :], a[:], bt[:], 0.0,
                                             ALU.mult, ALU.add)
            q2 = qpool.tile([128, 2, L], BF16, tag="q", bufs=2)
            nc.vector.tensor_tensor(out=q2[:], in0=h2[:], in1=cb2[:], op=ALU.mult)
            for i in (0, 1):
                for f in range(FT):
                    nc.tensor.matmul(py[:, f * FC:(f + 1) * FC], ident[:],
                                     q2[:, i, f * FC:(f + 1) * FC],
                                     start=False, stop=(n0 + i == D_STATE - 1))
        yfc = scanpool.tile([128, L], BF16, tag="yfc", bufs=1)
        yf_view = yfc[:] if p == "f" else yfc[:, ::-1]
        nc.vector.tensor_tensor(out=yf_view, in0=py[:], in1=zst[:], op=ALU.mult)
        nc.sync.dma_start(S["yf_d"][:, d * L:(d + 1) * L], yfc[:])

    def teardown_S(S):
        S["sctx"].close()

    def setup_O(S, octx_host):
        p = S["p"]
        octx = octx_host.enter_context(ExitStack())
        S["psO"] = octx.enter_context(tc.tile_pool(name=f"psO{p}", bufs=4, space="PSUM"))
        lnpool = octx.enter_context(tc.tile_pool(name=f"ln{p}", bufs=2))
        S["lnpool"] = lnpool
        wout = S["aw"].tile([128, NB * D_MODEL], BF16, tag="wout", name=f"wout_{p}")
        nc.sync.dma_start(wout[:], io[f"{p}_Wout"])
        S["wout"] = wout
        S["ympool"] = octx.enter_context(tc.tile_pool(name=f"ym{p}", bufs=3))

    def emit_O_mt(S, mt):
        p = S["p"]
        lnpool, wout = S["lnpool"], S["wout"]
        ym = S["ympool"].tile([128, NB, 128], BF16, tag="ym")
        ysrc = S["yf_d"].rearrange("p (k l) -> p k l", k=NB)[:, :, mt * 128:(mt + 1) * 128]
        nc.sync.dma_start(ym[:], ysrc)
        po = S["psO"].tile([128, D_MODEL], F32, tag="po")
        for k in range(NB):
            nc.tensor.matmul(po[:], ym[:, k, :],
                             wout[:, k * D_MODEL:(k + 1) * D_MODEL],
                             start=(k == 0), stop=(k == NB - 1))
        if p == "f":
            st = lnpool.tile([128, D_MODEL], F32, tag="st")
            nc.scalar.copy(st[:], po[:])
            nc.sync.dma_start(s_d[:, mt * D_MODEL:(mt + 1) * D_MODEL], st[:])
        else:
            sf = lnpool.tile([128, D_MODEL], F32, tag="sf")
            nc.sync.dma_start(sf[:], s_d[:, mt * D_MODEL:(mt + 1) * D_MODEL])
            s = lnpool.tile([128, D_MODEL], F32, tag="s")
            ssum = lnpool.tile([128, 1], F32, tag="ssum")
            nc.vector.tensor_tensor(out=s[:], in0=sf[:], in1=po[:], op=ALU.add)
            sdummy = lnpool.tile([128, D_MODEL], F32, tag="sdummy")
            nc.scalar.activation(sdummy[:], s[:], AF.Copy, accum_out=ssum[:])
            nmu = lnpool.tile([128, 1], F32, tag="nmu")
            nc.vector.tensor_scalar_mul(nmu[:], ssum[:], -1.0 / D_MODEL)
            sq = lnpool.tile([128, D_MODEL], F32, tag="sq")
            vsum = lnpool.tile([128, 1], F32, tag="vsum")
            nc.scalar.activation(sq[:], s[:], AF.Square, bias=nmu[:],
                                 accum_out=vsum[:])
            var = lnpool.tile([128, 1], F32, tag="var")
            nc.vector.tensor_scalar(out=var[:], in0=vsum[:],
                                    scalar1=1.0 / D_MODEL, scalar2=LN_EPS,
                                    op0=ALU.mult, op1=ALU.add)
            # rstd = 1/sqrt(var): fast-inverse-sqrt + 2 Newton steps on DVE
            # (avoids ACT sqrt, whose table would thrash against the scan exps)
            vi = var[:].bitcast(mybir.dt.int32)
            sh = lnpool.tile([128, 1], mybir.dt.int32, tag="sh")
            nc.vector.tensor_scalar(out=sh[:], in0=vi, scalar1=1, scalar2=None,
                                    op0=ALU.logical_shift_right)
            y0i = lnpool.tile([128, 1], mybir.dt.int32, tag="y0i")
            nc.vector.tensor_scalar(out=y0i[:], in0=sh[:], scalar1=-1,
                                    scalar2=0x5f3759df, op0=ALU.mult, op1=ALU.add)
            y = y0i[:].bitcast(F32)
            for it in range(2):
                yy = lnpool.tile([128, 1], F32, tag=f"yy{it}")
                nc.vector.tensor_tensor(out=yy[:], in0=y, in1=y, op=ALU.mult)
                vyy = lnpool.tile([128, 1], F32, tag=f"vyy{it}")
                nc.vector.tensor_tensor(out=vyy[:], in0=var[:], in1=yy[:], op=ALU.mult)
                hf = lnpool.tile([128, 1], F32, tag=f"hf{it}")
                nc.vector.tensor_scalar(out=hf[:], in0=vyy[:], scalar1=-0.5,
                                        scalar2=1.5, op0=ALU.mult, op1=ALU.add)
                yn = lnpool.tile([128, 1], F32, tag=f"yn{it}")
                nc.vector.tensor_tensor(out=yn[:], in0=hf[:], in1=y, op=ALU.mult)
                y = yn[:]
            xm = lnpool.tile([128, D_MODEL], F32, tag="xm")
            nc.vector.tensor_scalar(out=xm[:], in0=s[:], scalar1=nmu[:],
                                    scalar2=y, op0=ALU.add, op1=ALU.mult)
            o1 = lnpool.tile([128, D_MODEL], F32, tag="o1")
            nc.vector.tensor_tensor(out=o1[:], in0=xm[:], in1=ln_g[:], op=ALU.mult)
            o2 = lnpool.tile([128, D_MODEL], F32, tag="o2")
            nc.vector.tensor_tensor(out=o2[:], in0=o1[:], in1=ln_b[:], op=ALU.add)
            nc.sync.dma_start(io["out"][mt * 128:(mt + 1) * 128, :], o2[:])

    # ------------------------- schedule -------------------------
    dctx_f = ctx.enter_context(ExitStack())
    Sf = setup_dir("f", dctx_f)
    dctx_b = ctx.enter_context(ExitStack())
    Sb = setup_dir("b", dctx_b)
    alloc_uc(Sf)
    for _ in emit_A_steps(Sf, lean=False):
        pass
    emit_B(Sf)
    # f scan with b phase-A interleaved (2 A-units per d-block)
    setup_S(Sf)
    gA = emit_A_steps(Sb, lean=True)
    for d in range(NB):
        emit_S_d(Sf, d)
        next(gA, None)
        next(gA, None)
    for _ in gA:
        pass
    teardown_S(Sf)
    Sf["ucdt_ctx"].close()
    alloc_uc(Sb)
    load_uc(Sb)
    emit_B(Sb)
    # b scan with f out-projection interleaved
    setup_O(Sf, dctx_b)
    setup_S(Sb)
    for d in range(NB):
        emit_S_d(Sb, d)
        emit_O_mt(Sf, 2 * d)
        emit_O_mt(Sf, 2 * d + 1)
    teardown_S(Sb)
    # b out-projection + layernorm
    setup_O(Sb, dctx_b)
    for mt in range(MT):
        emit_O_mt(Sb, mt)


def build_nc(L: int) -> tuple[bass.Bass, dict]:
    nc = bacc.Bacc("TRN2", target_bir_lowering=False, debug=False)
    io = declare_ios(nc, L)
    with tile.TileContext(nc) as tc:
        with ExitStack() as ctx:
            build_kernel(ctx, tc, io, L)
    nc.compile()
    return nc, io


# ----------------------------------------------------------------------------
# kernel entry point
# ----------------------------------------------------------------------------
_CACHE = {}


def _get_nc(L: int):
    if L not in _CACHE:
        _CACHE[L] = build_nc(L)
    return _CACHE[L]


def kernel(**inputs) -> np.ndarray:
    from concourse.bass_utils import run_bass_kernel_spmd

    in_maps, L = host_prep(inputs)
    nc, io = _get_nc(L)
    n = len(in_maps)
    res = run_bass_kernel_spmd(nc, in_maps, core_ids=list(range(n)))
    return np.stack([np.asarray(res.results[i]["out"], dtype=np.float32) for i in range(n)])


def kernel_timed(reps: int = 5, **inputs):
    """Run on hardware with device-resident inputs; returns (out, best_ns).

    best_ns is the minimum wall-clock of a full 8-core dispatch (includes
    PJRT/axon launch overhead, so it upper-bounds device exec time).
    """
    import time
    import jax
    from jax.sharding import Mesh, PartitionSpec
    from jax.experimental.shard_map import shard_map
    from concourse import bass2jax as b2j

    in_maps, L = host_prep(inputs)
    nc, io = _get_nc(L)
    n_cores = len(in_maps)
    b2j.install_neuronx_cc_hook()

    part_name = nc.partition_id_tensor.name if nc.partition_id_tensor else None
    in_names, out_names, out_avals, zero_outs = [], [], [], []
    for alloc in nc.m.functions[0].allocations:
        if not isinstance(alloc, mybir.MemoryLocationSet):
            continue
        name = alloc.memorylocations[0].name
        if alloc.kind == "ExternalInput":
            if name != part_name:
                in_names.append(name)
        elif alloc.kind == "ExternalOutput":
            out_names.append(name)
            shp = list(alloc.tensor_shape)
            npdt = mybir.dt.np(alloc.dtype)
            out_avals.append(jax.core.ShapedArray(shp, npdt))
            zero_outs.append(np.zeros(shp, npdt))
    n_params = len(in_names)
    n_outs = len(out_names)
    all_in_names = in_names + out_names
    if part_name is not None:
        all_in_names = all_in_names + [part_name]

    def _body(*args):
        operands = list(args)
        if part_name is not None:
            operands.append(b2j.partition_id_tensor())
        outs = b2j._bass_exec_p.bind(
            *operands, out_avals=tuple(out_avals), in_names=tuple(all_in_names),
            out_names=tuple(out_names), lowering_input_output_aliases=(),
            sim_require_finite=True, sim_require_nnan=True, nc=nc)
        return tuple(outs)

    devices = jax.devices()[:n_cores]
    mesh = Mesh(np.asarray(devices), ("core",))
    in_specs = (PartitionSpec("core"),) * (n_params + n_outs)
    out_specs = (PartitionSpec("core"),) * n_outs
    sharded = jax.jit(shard_map(_body, mesh=mesh, in_specs=in_specs,
                                out_specs=out_specs, check_rep=False),
                      keep_unused=True)
    concat_in = [np.concatenate([np.asarray(m[nm]) for m in in_maps], axis=0)
                 for nm in in_names]
    concat_zeros = [np.zeros((n_cores * z.shape[0], *z.shape[1:]), z.dtype)
                    for z in zero_outs]
    from jax.sharding import NamedSharding
    shard = NamedSharding(mesh, PartitionSpec("core"))
    dev_in = [jax.device_put(a, shard) for a in concat_in]
    dev_zero = [jax.device_put(a, shard) for a in concat_zeros]

    out_arrs = sharded(*dev_in, *dev_zero)           # warmup/compile
    jax.block_until_ready(out_arrs)
    # Per-dispatch wall time is dominated by the axon/PJRT tunnel (~75 ms for
    # a trivial kernel).  Estimate device exec time from the marginal cost of
    # pipelined async dispatches: (T(N2) - T(N1)) / (N2 - N1), which hides
    # the per-call launch overhead (a trivial kernel measures ~0.6 ms here).
    def timed(n):
        t0 = time.perf_counter()
        rs = [sharded(*dev_in, *dev_zero) for _ in range(n)]
        jax.block_until_ready(rs)
        return time.perf_counter() - t0
    n1, n2 = 16, 64
    t1s, t2s = [], []
    for _ in range(max(2, reps)):
        t1s.append(timed(n1))
        t2s.append(timed(n2))
    best = (min(t2s) - min(t1s)) / (n2 - n1)
    out = np.stack([
        np.asarray(out_arrs[0]).reshape(n_cores, *out_avals[0].shape)[c]
        for c in range(n_cores)
    ]).astype(np.float32)
    return out, best * 1e9


if __name__ == "__main__":
    import time
    npz = np.load("/tmp/inputs.npz")
    inputs = {k: npz[k] for k in npz.files}
    t0 = time.time()
    out = kernel(**inputs)
    print(f"kernel done in {time.time()-t0:.1f}s, out shape {out.shape}")
    out2, ns = kernel_timed(**inputs)
    print(f"timed: {ns:.0f} ns  ({ns/1e6:.3f} ms)")



# revision 60
# speedup vs baseline: 1.0192x; 1.0192x over previous
"""BiMamba block (bidirectional Mamba-1 + layernorm) as a Bass/Tile kernel
for 8 Trainium2 NeuronCores.

Sharding: data-parallel over batch — core i computes batch row i end-to-end
(both scan directions + layernorm), no collectives.

Per-core layout: channel-major [channel(partition), time(free)] activations
until the output projection, which emits token-major [token, d_model].

The depthwise conv and the D-skip connection run on the PE as diag(w)/diag(D)
stationary matmuls (the D-skip opens the same PSUM accumulation group that the
per-state ident matmuls accumulate into), freeing DVE cycles for the scans.

Selective scan per (d-block of 128 channels, state index n) on [128, L] tiles:
    a = exp(A[:,n] * dt)            ACT, per-partition scale AP
    b = (dt*u) * B_bcast[n]         DVE tensor_tensor bf16 (2x mode)
    h = tensor_tensor_scan(a, b)    DVE, fp32 internal recurrence state
    q = h * C_bcast[n]              DVE tensor_tensor bf16 (2x mode)
    psum_y += I @ q                 PE accumulates the sum over n in PSUM

B/C broadcasts are DMA re-reads of a small DRAM staging row with a
partition-step-0 access pattern.  The backward direction runs on
host-reversed input; un-reversal is free via a negative-stride output AP at
the yf write.
"""

import os
import sys
from contextlib import ExitStack

for _p in ("/opt/trn_rl_repo", "/root/.axon_site/_ro/trn_rl_repo"):
    if os.path.isdir(_p) and _p not in sys.path:
        sys.path.insert(0, _p)

import numpy as np
import ml_dtypes

import concourse.bass as bass
import concourse.tile as tile
from concourse import bacc, mybir
from concourse.masks import make_identity

AF = mybir.ActivationFunctionType
ALU = mybir.AluOpType
F32 = mybir.dt.float32
F32R = mybir.dt.float32r
BF16 = mybir.dt.bfloat16
F16 = mybir.dt.float16

D_MODEL = 512
D_STATE = 16
D_CONV = 4
D_INNER = 1024
DT_RANK = 32
NB = D_INNER // 128          # 8 d-blocks
KM = D_MODEL // 128          # 4 k-tiles over d_model
LN_EPS = 1e-5

XZ_F32R = False              # xz matmul in float32r (else bf16)
SCAN_B_BF16 = True           # scan data1 dtype bf16 (else fp32)
STOP_AFTER = os.environ.get("BIMAMBA_STOP_AFTER", "")


def host_prep(inputs: dict, l_override: int | None = None) -> tuple[list[dict], int]:
    """Full problem inputs -> per-core in_maps (one batch row per core)."""
    x = np.asarray(inputs["x"], dtype=np.float32)
    Bsz, L, _ = x.shape
    if l_override is not None:
        L = l_override
        x = x[:, :L]
    bf = ml_dtypes.bfloat16

    def pack(a, nblk):  # [nblk*128, F] -> [128, nblk*F]
        return np.concatenate([a[i * 128:(i + 1) * 128] for i in range(nblk)], axis=1).copy()

    shared = {}
    for p in ("f", "b"):
        Wxz = np.asarray(inputs[f"{p}_Wxz"], np.float32)
        shared[f"{p}_Wxz"] = pack(Wxz, KM) if XZ_F32R else pack(Wxz, KM).astype(bf)
        cw = np.asarray(inputs[f"{p}_conv_w"], np.float32).reshape(D_INNER, D_CONV)
        shared[f"{p}_convw"] = pack(cw, NB)
        cb = np.asarray(inputs[f"{p}_conv_b"], np.float32).reshape(D_INNER, 1)
        shared[f"{p}_convb"] = pack(cb, NB)
        Wxm = np.asarray(inputs[f"{p}_Wx"], np.float32)
        Wxp = np.zeros((D_INNER, 80), np.float32)
        Wxp[:, 0:48] = Wxm[:, 0:48]
        Wxp[:, 64:80] = Wxm[:, 48:64]
        shared[f"{p}_Wx"] = pack(Wxp, NB).astype(bf)
        Wdtf = np.asarray(inputs[f"{p}_Wdt"], np.float32)
        Wdt_hi = Wdtf.astype(bf)
        Wdt_lo = (Wdtf - Wdt_hi.astype(np.float32)).astype(bf)
        shared[f"{p}_Wdt"] = np.concatenate([Wdt_hi, Wdt_lo], axis=1).copy()
        bdt = np.asarray(inputs[f"{p}_bdt"], np.float32).reshape(D_INNER, 1)
        shared[f"{p}_bdt"] = pack(bdt, NB)
        # positive-dt convention: dtt = softplus(...), exp scale = A (negative)
        negA = -np.exp(np.asarray(inputs[f"{p}_A_log"], np.float32))
        shared[f"{p}_A"] = pack(negA, NB)
        Dv = np.asarray(inputs[f"{p}_D"], np.float32).reshape(D_INNER, 1)
        shared[f"{p}_D"] = pack(Dv, NB)
        shared[f"{p}_Wout"] = pack(np.asarray(inputs[f"{p}_Wout"], np.float32), NB).astype(bf)
    shared["ln_g"] = np.broadcast_to(np.asarray(inputs["ln_g"], np.float32)[None, :], (128, D_MODEL)).copy()
    shared["ln_b"] = np.broadcast_to(np.asarray(inputs["ln_b"], np.float32)[None, :], (128, D_MODEL)).copy()

    in_maps = []
    for bi in range(Bsz):
        xT = np.ascontiguousarray(x[bi].T)        # [512, L]
        xTr = np.ascontiguousarray(x[bi][::-1].T)
        m = dict(shared)
        m["xT"] = pack(xT, KM) if XZ_F32R else pack(xT, KM).astype(bf)
        m["xTr"] = pack(xTr, KM) if XZ_F32R else pack(xTr, KM).astype(bf)
        in_maps.append(m)
    return in_maps, L


def declare_ios(nc: bass.Bass, L: int) -> dict:
    io = {}
    xdt = F32R if XZ_F32R else BF16
    io["xT"] = nc.dram_tensor("xT", [128, KM * L], xdt, kind="ExternalInput").ap()
    io["xTr"] = nc.dram_tensor("xTr", [128, KM * L], xdt, kind="ExternalInput").ap()
    for p in ("f", "b"):
        io[f"{p}_Wxz"] = nc.dram_tensor(f"{p}_Wxz", [128, KM * 2 * D_INNER], xdt, kind="ExternalInput").ap()
        io[f"{p}_convw"] = nc.dram_tensor(f"{p}_convw", [128, NB * D_CONV], F32, kind="ExternalInput").ap()
        io[f"{p}_convb"] = nc.dram_tensor(f"{p}_convb", [128, NB], F32, kind="ExternalInput").ap()
        io[f"{p}_Wx"] = nc.dram_tensor(f"{p}_Wx", [128, NB * 80], BF16, kind="ExternalInput").ap()
        io[f"{p}_Wdt"] = nc.dram_tensor(f"{p}_Wdt", [32, 2 * D_INNER], BF16, kind="ExternalInput").ap()
        io[f"{p}_bdt"] = nc.dram_tensor(f"{p}_bdt", [128, NB], F32, kind="ExternalInput").ap()
        io[f"{p}_A"] = nc.dram_tensor(f"{p}_A", [128, NB * D_STATE], F32, kind="ExternalInput").ap()
        io[f"{p}_D"] = nc.dram_tensor(f"{p}_D", [128, NB], F32, kind="ExternalInput").ap()
        io[f"{p}_Wout"] = nc.dram_tensor(f"{p}_Wout", [128, NB * D_MODEL], BF16, kind="ExternalInput").ap()
    io["ln_g"] = nc.dram_tensor("ln_g", [128, D_MODEL], F32, kind="ExternalInput").ap()
    io["ln_b"] = nc.dram_tensor("ln_b", [128, D_MODEL], F32, kind="ExternalInput").ap()
    io["out"] = nc.dram_tensor("out", [L, D_MODEL], F32, kind="ExternalOutput").ap()
    return io


def build_kernel(ctx: ExitStack, tc: tile.TileContext, io: dict, L: int):
    nc = tc.nc
    FC = min(512, L)
    FT = L // FC                 # 512-wide free chunks
    MT = L // 128                # token tiles
    HM = D_INNER // 128          # m-tiles per xz half (8)

    wpool = ctx.enter_context(tc.tile_pool(name="wglob", bufs=1))
    ident = wpool.tile([128, 128], BF16, tag="ident")
    make_identity(nc, ident[:])
    ln_g = wpool.tile([128, D_MODEL], F32, tag="ln_g")
    nc.sync.dma_start(ln_g[:], io["ln_g"])
    ln_b = wpool.tile([128, D_MODEL], F32, tag="ln_b")
    nc.sync.dma_start(ln_b[:], io["ln_b"])
    dglob = ctx.enter_context(tc.tile_pool(name="dglob", bufs=1, space="DRAM"))
    s_d = dglob.tile([128, MT * D_MODEL], F32, tag="s_d")

    def setup_dir(p, dctx):
        S = {"p": p, "dctx": dctx}
        aw = dctx.enter_context(tc.tile_pool(name=f"aw{p}", bufs=1))
        S["aw"] = aw
        dpool = dctx.enter_context(tc.tile_pool(name=f"dram{p}", bufs=1, space="DRAM"))
        S["zs_d"] = dpool.tile([128, NB * L], BF16, tag="zs_d", name=f"zs_d_{p}")
        S["bc_d"] = dpool.tile([32, L], BF16, tag="bc_d", name=f"bc_d_{p}")
        S["uc_d"] = dpool.tile([128, NB * L], BF16, tag="uc_d", name=f"uc_d_{p}")
        S["yf_d"] = dpool.tile([128, NB * L], BF16, tag="yf_d", name=f"yf_d_{p}")
        S["wx"] = aw.tile([128, NB * 80], BF16, tag="wx", name=f"wx_{p}")
        nc.sync.dma_start(S["wx"][:], io[f"{p}_Wx"])
        S["amat"] = aw.tile([128, NB * D_STATE], F32, tag="amat", name=f"amat_{p}")
        nc.sync.dma_start(S["amat"][:], io[f"{p}_A"])
        dmat = aw.tile([128, NB], F32, tag="dmat")
        nc.sync.dma_start(dmat[:], io[f"{p}_D"])
        convw_g = aw.tile([128, NB * D_CONV], F32, tag="convw_g", name=f"convw_g_{p}")
        nc.sync.dma_start(convw_g[:], io[f"{p}_convw"])
        S["convw_g"] = convw_g
        S["convb"] = aw.tile([128, NB], F32, tag="convb", name=f"convb_{p}")
        nc.sync.dma_start(S["convb"][:], io[f"{p}_convb"])
        diagd = [aw.tile([128, 128], BF16, tag=f"dgd{d}", name=f"dgd{d}_{p}")
                 for d in range(NB)]
        for d in range(NB):
            nc.vector.tensor_scalar_mul(diagd[d][:], ident[:], dmat[:, d:d + 1])
        S["diagd"] = diagd
        return S

    def emit_A_steps(S, lean):
        """Phase A: xz matmul, conv on PE, silu.

        lean=True (overlapped direction): small buffers, uc staged to DRAM.
        lean=False: full buffers, silu written straight into SBUF uc tiles.
        Generator yielding after each (half, m8) unit (16 yields)."""
        p = S["p"]
        with ExitStack() as actx:
            apool = actx.enter_context(tc.tile_pool(name=f"pa{p}", bufs=1))
            whpool = actx.enter_context(tc.tile_pool(name=f"wh{p}", bufs=1 if lean else 2))
            u0pool = actx.enter_context(tc.tile_pool(name=f"u0p{p}", bufs=2 if lean else 3))
            dwpool = actx.enter_context(tc.tile_pool(name=f"dw{p}", bufs=2))
            evpool = actx.enter_context(tc.tile_pool(name=f"ev{p}", bufs=2 if lean else 3))
            psA = actx.enter_context(tc.tile_pool(name=f"psA{p}", bufs=1 if lean else 2, space="PSUM"))

            xin = apool.tile([128, KM * L], BF16, tag="xin")
            xsrc = io["xT" if p == "f" else "xTr"]
            for k in range(KM):
                nc.sync.dma_start(xin[:, k * L:(k + 1) * L], xsrc[:, k * L:(k + 1) * L])
            for half in range(2):        # 0: u-channels, 1: z-channels
                wh = whpool.tile([128, KM * D_INNER], BF16, tag="wh")
                wsrc = io[f"{p}_Wxz"].rearrange("p (k c) -> p k c", k=KM)[
                    :, :, half * D_INNER:(half + 1) * D_INNER]
                for k in range(KM):
                    nc.sync.dma_start(wh[:, k * D_INNER:(k + 1) * D_INNER], wsrc[:, k, :])
                for m8 in range(HM):
                    u0t = None
                    dgw = None
                    if half == 0:
                        u0t = u0pool.tile([128, D_CONV - 1 + L], BF16, tag="u0")
                        nc.gpsimd.memset(u0t[:, 0:D_CONV - 1], 0.0)
                        dgw = [dwpool.tile([128, 128], BF16, tag=f"dgw{j}",
                                           name=f"dgw{p}_{m8}_{j}") for j in range(D_CONV)]
                        for j in range(D_CONV):
                            nc.vector.tensor_scalar_mul(
                                dgw[j][:], ident[:],
                                S["convw_g"][:, m8 * D_CONV + j: m8 * D_CONV + j + 1])
                    for fp in range(FT // 2):
                        # k-outer over an f-pair: each stationary k-slice
                        # serves 2 matmuls back-to-back (same-stationary
                        # matmuls are ~2x cheaper on HW)
                        pss = [psA.tile([128, FC], F32, tag=f"pxz{i}",
                                        name=f"pxz{p}_{half}_{m8}_{fp}_{i}")
                               for i in range(2)]
                        for k in range(KM):
                            for i in range(2):
                                f = 2 * fp + i
                                nc.tensor.matmul(
                                    pss[i][:],
                                    wh[:, k * D_INNER + m8 * 128: k * D_INNER + (m8 + 1) * 128],
                                    xin[:, k * L + f * FC: k * L + (f + 1) * FC],
                                    start=(k == 0), stop=(k == KM - 1),
                                )
                        for i in range(2):
                            f = 2 * fp + i
                            if half == 0:
                                nc.scalar.copy(
                                    u0t[:, D_CONV - 1 + f * FC: D_CONV - 1 + (f + 1) * FC],
                                    pss[i][:])
                            else:
                                zt = evpool.tile([128, FC], BF16, tag="zt")
                                if lean:
                                    nc.scalar.copy(zt[:], pss[i][:])   # raw z; silu later
                                else:
                                    nc.scalar.activation(zt[:], pss[i][:], AF.Silu)
                                nc.sync.dma_start(
                                    S["zs_d"][:, m8 * L + f * FC: m8 * L + (f + 1) * FC], zt[:])
                    if half == 0:
                        # depthwise causal conv via diag(w_j) matmuls,
                        # j-outer over f-pairs for stationary reuse
                        for fp in range(FT // 2):
                            pcs = [psA.tile([128, FC], F32, tag=f"pcv{i}",
                                            name=f"pcv{p}_{m8}_{fp}_{i}")
                                   for i in range(2)]
                            for j in range(D_CONV):
                                for i in range(2):
                                    f = 2 * fp + i
                                    nc.tensor.matmul(
                                        pcs[i][:], dgw[j][:],
                                        u0t[:, f * FC + j: f * FC + j + FC],
                                        start=(j == 0), stop=(j == D_CONV - 1))
                            for i in range(2):
                                f = 2 * fp + i
                                if lean:
                                    # identity+bias copy; silu deferred to load_uc
                                    ut = evpool.tile([128, FC], BF16, tag="ut")
                                    nc.scalar.activation(ut[:], pcs[i][:], AF.Identity,
                                                         bias=S["convb"][:, m8:m8 + 1])
                                    nc.sync.dma_start(
                                        S["uc_d"][:, m8 * L + f * FC: m8 * L + (f + 1) * FC], ut[:])
                                else:
                                    nc.scalar.activation(
                                        S["uc"][m8][:, f * FC:(f + 1) * FC], pcs[i][:],
                                        AF.Silu, bias=S["convb"][:, m8:m8 + 1])
                    yield

    def alloc_uc(S):
        p = S["p"]
        S["ucdt_ctx"] = S["dctx"].enter_context(ExitStack())
        ucpool = S["ucdt_ctx"].enter_context(tc.tile_pool(name=f"ucp{p}", bufs=1))
        S["uc"] = [ucpool.tile([128, L], BF16, tag=f"uc{d}", name=f"uc{d}_{p}")
                   for d in range(NB)]

    def load_uc(S):
        for d in range(NB):
            nc.sync.dma_start(S["uc"][d][:], S["uc_d"][:, d * L:(d + 1) * L])
        zpool = S["ucdt_ctx"].enter_context(tc.tile_pool(name=f"zsil{S['p']}", bufs=2))
        for d in range(NB):
            nc.scalar.activation(S["uc"][d][:], S["uc"][d][:], AF.Silu)
            zr = zpool.tile([128, L], BF16, tag="zr")
            nc.sync.dma_start(zr[:], S["zs_d"][:, d * L:(d + 1) * L])
            nc.scalar.activation(zr[:], zr[:], AF.Silu)
            nc.sync.dma_start(S["zs_d"][:, d * L:(d + 1) * L], zr[:])

    def emit_B(S):
        p = S["p"]
        uc = S["uc"]
        dtpool = S["ucdt_ctx"].enter_context(tc.tile_pool(name=f"dtp{p}", bufs=1))
        S["dtt"] = [dtpool.tile([128, L], F16, tag=f"dt{d}", name=f"dt{d}_{p}")
                    for d in range(NB)]
        dtt = S["dtt"]
        with ExitStack() as bctx:
            bpool = bctx.enter_context(tc.tile_pool(name=f"pb{p}", bufs=1))
            psB = bctx.enter_context(tc.tile_pool(name=f"psB{p}", bufs=1, space="PSUM"))
            wdt = bpool.tile([32, 2 * D_INNER], BF16, tag="wdt")
            nc.sync.dma_start(wdt[:], io[f"{p}_Wdt"])
            bdt = bpool.tile([128, NB], F32, tag="bdt")
            nc.sync.dma_start(bdt[:], io[f"{p}_bdt"])
            dtr = bpool.tile([32, L], F32, tag="dtr")
            dtr_hi = bpool.tile([32, L], BF16, tag="dtr_hi")
            dtr_lo = bpool.tile([32, L], BF16, tag="dtr_lo")
            bmr = bpool.tile([16, L], BF16, tag="bmr")
            cmr = bpool.tile([16, L], BF16, tag="cmr")
            pxs = [psB.tile([80, FC], F32, tag=f"pxd{f}", name=f"pxd{p}_{f}", bufs=1)
                   for f in range(FT)]
            for k in range(NB):
                for f in range(FT):
                    nc.tensor.matmul(pxs[f][:], S["wx"][:, k * 80:(k + 1) * 80],
                                     uc[k][:, f * FC:(f + 1) * FC],
                                     start=(k == 0), stop=(k == NB - 1))
            for f in range(FT):
                ps = pxs[f]
                nc.scalar.copy(dtr[:, f * FC:(f + 1) * FC], ps[0:DT_RANK, :])
                nc.scalar.copy(dtr_hi[:, f * FC:(f + 1) * FC], ps[0:DT_RANK, :])
                nc.scalar.copy(bmr[:, f * FC:(f + 1) * FC], ps[DT_RANK:DT_RANK + D_STATE, :])
                nc.scalar.copy(cmr[:, f * FC:(f + 1) * FC], ps[64:80, :])
            nc.sync.dma_start(S["bc_d"][0:16, :], bmr[:])
            nc.sync.dma_start(S["bc_d"][16:32, :], cmr[:])
            nc.vector.tensor_tensor(out=dtr_lo[:], in0=dtr[:], in1=dtr_hi[:],
                                    op=ALU.subtract)
            # dtt = softplus(dtproj + bdt) = ln(exp(dtproj + bdt) + 1).
            # exp chunks all batched first, then the lns: the compiler maps
            # exp and ln to different act tables, so interleaving them costs
            # a 1283ns table load per switch.
            sg = [bpool.tile([128, L], F32, tag=f"sg{d}", name=f"sg{d}_{p}")
                  for d in range(NB)]
            for d in range(NB):
                # stationary-grouped: the hi slice serves 8 matmuls (4 chunks
                # x 2 movings) back-to-back, then the lo slice serves 4
                pds = [psB.tile([128, FC], F32, tag=f"pdt{f}",
                                name=f"pdt{p}_{d}_{f}") for f in range(FT)]
                for f in range(FT):
                    nc.tensor.matmul(pds[f][:], wdt[:, d * 128:(d + 1) * 128],
                                     dtr_hi[:, f * FC:(f + 1) * FC],
                                     start=True, stop=False)
                for f in range(FT):
                    nc.tensor.matmul(pds[f][:], wdt[:, d * 128:(d + 1) * 128],
                                     dtr_lo[:, f * FC:(f + 1) * FC],
                                     start=False, stop=False)
                for f in range(FT):
                    nc.tensor.matmul(pds[f][:], wdt[:, D_INNER + d * 128: D_INNER + (d + 1) * 128],
                                     dtr_hi[:, f * FC:(f + 1) * FC],
                                     start=False, stop=True)
                for f in range(FT):
                    nc.scalar.activation(sg[d][:, f * FC:(f + 1) * FC], pds[f][:],
                                         AF.Exp, bias=bdt[:, d:d + 1])
            for d in range(NB):
                nc.scalar.activation(dtt[d][:], sg[d][:], AF.Ln, bias=1.0)

    def setup_S(S):
        p = S["p"]
        sctx = ExitStack()
        S["sctx"] = sctx
        S["scanpool"] = sctx.enter_context(tc.tile_pool(name=f"sc{p}", bufs=2))
        S["qpool"] = sctx.enter_context(tc.tile_pool(name=f"q{p}", bufs=1))
        S["bcpool"] = sctx.enter_context(tc.tile_pool(name=f"bc{p}", bufs=2))
        S["psY"] = sctx.enter_context(tc.tile_pool(name=f"psY{p}", bufs=1, space="PSUM"))

    def emit_S_d(S, d):
        p = S["p"]
        scanpool, qpool, bcpool, psY = S["scanpool"], S["qpool"], S["bcpool"], S["psY"]
        uc, dtt, bc_d, zs_d, amat, diagd = (S["uc"], S["dtt"], S["bc_d"],
                                            S["zs_d"], S["amat"], S["diagd"])
        dtu = scanpool.tile([128, L], BF16, tag="dtu", bufs=1)
        nc.vector.tensor_tensor(out=dtu[:], in0=dtt[d][:], in1=uc[d][:], op=ALU.mult)
        zst = scanpool.tile([128, L], BF16, tag="zst", bufs=1)
        nc.sync.dma_start(zst[:], zs_d[:, d * L:(d + 1) * L])
        py = psY.tile([128, L], F32, tag="py")
        for f in range(FT):
            nc.tensor.matmul(py[:, f * FC:(f + 1) * FC], diagd[d][:],
                             uc[d][:, f * FC:(f + 1) * FC],
                             start=True, stop=False)
        for np2 in range(D_STATE // 2):
            n0 = 2 * np2
            cb2 = bcpool.tile([128, 2, L], BF16, tag="cb", bufs=2)
            nc.sync.dma_start(cb2[:, 0, :], bc_d[16 + n0:17 + n0, :].broadcast_to((128, L)))
            nc.sync.dma_start(cb2[:, 1, :], bc_d[17 + n0:18 + n0, :].broadcast_to((128, L)))
            h2 = scanpool.tile([128, 2, L], BF16, tag="h", bufs=1)
            for i in (0, 1):
                n = n0 + i
                a = scanpool.tile([128, L], F16, tag="a")
                nc.scalar.activation(a[:], dtt[d][:], AF.Exp,
                                     scale=amat[:, d * D_STATE + n: d * D_STATE + n + 1])
                bb = bcpool.tile([128, L], BF16, tag="bb")
                nc.sync.dma_start(bb[:], bc_d[n:n + 1, :].broadcast_to((128, L)))
                bt = scanpool.tile([128, L], BF16, tag="bt")
                nc.vector.tensor_tensor(out=bt[:], in0=dtu[:], in1=bb[:], op=ALU.mult)
                nc.vector.tensor_tensor_scan(h2[:, i, :], a[:], bt[:], 0.0,
                                             ALU.mult, ALU.add)
            q2 = qpool.tile([128, 2, L], BF16, tag="q", bufs=2)
            nc.vector.tensor_tensor(out=q2[:], in0=h2[:], in1=cb2[:], op=ALU.mult)
            for i in (0, 1):
                for f in range(FT):
                    nc.tensor.matmul(py[:, f * FC:(f + 1) * FC], ident[:],
                                     q2[:, i, f * FC:(f + 1) * FC],
                                     start=False, stop=(n0 + i == D_STATE - 1))
        yfc = scanpool.tile([128, L], BF16, tag="yfc", bufs=1)
        yf_view = yfc[:] if p == "f" else yfc[:, ::-1]
        nc.vector.tensor_tensor(out=yf_view, in0=py[:], in1=zst[:], op=ALU.mult)
        nc.sync.dma_start(S["yf_d"][:, d * L:(d + 1) * L], yfc[:])

    def teardown_S(S):
        S["sctx"].close()

    def setup_O(S, octx_host):
        p = S["p"]
        octx = octx_host.enter_context(ExitStack())
        S["psO"] = octx.enter_context(tc.tile_pool(name=f"psO{p}", bufs=4, space="PSUM"))
        lnpool = octx.enter_context(tc.tile_pool(name=f"ln{p}", bufs=2))
        S["lnpool"] = lnpool
        wout = S["aw"].tile([128, NB * D_MODEL], BF16, tag="wout", name=f"wout_{p}")
        nc.sync.dma_start(wout[:], io[f"{p}_Wout"])
        S["wout"] = wout
        S["ympool"] = octx.enter_context(tc.tile_pool(name=f"ym{p}", bufs=3))

    def emit_O_mt(S, mt):
        p = S["p"]
        lnpool, wout = S["lnpool"], S["wout"]
        ym = S["ympool"].tile([128, NB, 128], BF16, tag="ym")
        ysrc = S["yf_d"].rearrange("p (k l) -> p k l", k=NB)[:, :, mt * 128:(mt + 1) * 128]
        nc.sync.dma_start(ym[:], ysrc)
        po = S["psO"].tile([128, D_MODEL], F32, tag="po")
        for k in range(NB):
            nc.tensor.matmul(po[:], ym[:, k, :],
                             wout[:, k * D_MODEL:(k + 1) * D_MODEL],
                             start=(k == 0), stop=(k == NB - 1))
        if p == "f":
            st = lnpool.tile([128, D_MODEL], F32, tag="st")
            nc.scalar.copy(st[:], po[:])
            nc.sync.dma_start(s_d[:, mt * D_MODEL:(mt + 1) * D_MODEL], st[:])
        else:
            sf = lnpool.tile([128, D_MODEL], F32, tag="sf")
            nc.sync.dma_start(sf[:], s_d[:, mt * D_MODEL:(mt + 1) * D_MODEL])
            s = lnpool.tile([128, D_MODEL], F32, tag="s")
            ssum = lnpool.tile([128, 1], F32, tag="ssum")
            nc.vector.tensor_tensor(out=s[:], in0=sf[:], in1=po[:], op=ALU.add)
            sdummy = lnpool.tile([128, D_MODEL], F32, tag="sdummy")
            nc.scalar.activation(sdummy[:], s[:], AF.Copy, accum_out=ssum[:])
            nmu = lnpool.tile([128, 1], F32, tag="nmu")
            nc.vector.tensor_scalar_mul(nmu[:], ssum[:], -1.0 / D_MODEL)
            sq = lnpool.tile([128, D_MODEL], F32, tag="sq")
            vsum = lnpool.tile([128, 1], F32, tag="vsum")
            nc.scalar.activation(sq[:], s[:], AF.Square, bias=nmu[:],
                                 accum_out=vsum[:])
            var = lnpool.tile([128, 1], F32, tag="var")
            nc.vector.tensor_scalar(out=var[:], in0=vsum[:],
                                    scalar1=1.0 / D_MODEL, scalar2=LN_EPS,
                                    op0=ALU.mult, op1=ALU.add)
            # rstd = 1/sqrt(var): fast-inverse-sqrt + 2 Newton steps on DVE
            # (avoids ACT sqrt, whose table would thrash against the scan exps)
            vi = var[:].bitcast(mybir.dt.int32)
            sh = lnpool.tile([128, 1], mybir.dt.int32, tag="sh")
            nc.vector.tensor_scalar(out=sh[:], in0=vi, scalar1=1, scalar2=None,
                                    op0=ALU.logical_shift_right)
            y0i = lnpool.tile([128, 1], mybir.dt.int32, tag="y0i")
            nc.vector.tensor_scalar(out=y0i[:], in0=sh[:], scalar1=-1,
                                    scalar2=0x5f3759df, op0=ALU.mult, op1=ALU.add)
            y = y0i[:].bitcast(F32)
            for it in range(2):
                yy = lnpool.tile([128, 1], F32, tag=f"yy{it}")
                nc.vector.tensor_tensor(out=yy[:], in0=y, in1=y, op=ALU.mult)
                vyy = lnpool.tile([128, 1], F32, tag=f"vyy{it}")
                nc.vector.tensor_tensor(out=vyy[:], in0=var[:], in1=yy[:], op=ALU.mult)
                hf = lnpool.tile([128, 1], F32, tag=f"hf{it}")
                nc.vector.tensor_scalar(out=hf[:], in0=vyy[:], scalar1=-0.5,
                                        scalar2=1.5, op0=ALU.mult, op1=ALU.add)
                yn = lnpool.tile([128, 1], F32, tag=f"yn{it}")
                nc.vector.tensor_tensor(out=yn[:], in0=hf[:], in1=y, op=ALU.mult)
                y = yn[:]
            xm = lnpool.tile([128, D_MODEL], F32, tag="xm")
            nc.vector.tensor_scalar(out=xm[:], in0=s[:], scalar1=nmu[:],
                                    scalar2=y, op0=ALU.add, op1=ALU.mult)
            o1 = lnpool.tile([128, D_MODEL], F32, tag="o1")
            nc.vector.tensor_tensor(out=o1[:], in0=xm[:], in1=ln_g[:], op=ALU.mult)
            o2 = lnpool.tile([128, D_MODEL], F32, tag="o2")
            nc.vector.tensor_tensor(out=o2[:], in0=o1[:], in1=ln_b[:], op=ALU.add)
            nc.sync.dma_start(io["out"][mt * 128:(mt + 1) * 128, :], o2[:])

    # ------------------------- schedule -------------------------
    dctx_f = ctx.enter_context(ExitStack())
    Sf = setup_dir("f", dctx_f)
    dctx_b = ctx.enter_context(ExitStack())
    Sb = setup_dir("b", dctx_b)
    alloc_uc(Sf)
    for _ in emit_A_steps(Sf, lean=False):
        pass
    emit_B(Sf)
    # f scan with b phase-A interleaved (2 A-units per d-block)
    setup_S(Sf)
    gA = emit_A_steps(Sb, lean=True)
    for d in range(NB):
        emit_S_d(Sf, d)
        next(gA, None)
        next(gA, None)
    for _ in gA:
        pass
    teardown_S(Sf)
    Sf["ucdt_ctx"].close()
    alloc_uc(Sb)
    load_uc(Sb)
    emit_B(Sb)
    # b scan with f out-projection interleaved
    setup_O(Sf, dctx_b)
    setup_S(Sb)
    for d in range(NB):
        emit_S_d(Sb, d)
        emit_O_mt(Sf, 2 * d)
        emit_O_mt(Sf, 2 * d + 1)
    teardown_S(Sb)
    # b out-projection + layernorm
    setup_O(Sb, dctx_b)
    for mt in range(MT):
        emit_O_mt(Sb, mt)


def build_nc(L: int) -> tuple[bass.Bass, dict]:
    nc = bacc.Bacc("TRN2", target_bir_lowering=False, debug=False)
    io = declare_ios(nc, L)
    with tile.TileContext(nc) as tc:
        with ExitStack() as ctx:
            build_kernel(ctx, tc, io, L)
    nc.compile()
    return nc, io


# ----------------------------------------------------------------------------
# kernel entry point
# ----------------------------------------------------------------------------
_CACHE = {}


def _get_nc(L: int):
    if L not in _CACHE:
        _CACHE[L] = build_nc(L)
    return _CACHE[L]


def kernel(**inputs) -> np.ndarray:
    from concourse.bass_utils import run_bass_kernel_spmd

    in_maps, L = host_prep(inputs)
    nc, io = _get_nc(L)
    n = len(in_maps)
    res = run_bass_kernel_spmd(nc, in_maps, core_ids=list(range(n)))
    return np.stack([np.asarray(res.results[i]["out"], dtype=np.float32) for i in range(n)])


def kernel_timed(reps: int = 5, **inputs):
    """Run on hardware with device-resident inputs; returns (out, best_ns).

    best_ns is the minimum wall-clock of a full 8-core dispatch (includes
    PJRT/axon launch overhead, so it upper-bounds device exec time).
    """
    import time
    import jax
    from jax.sharding import Mesh, PartitionSpec
    from jax.experimental.shard_map import shard_map
    from concourse import bass2jax as b2j

    in_maps, L = host_prep(inputs)
    nc, io = _get_nc(L)
    n_cores = len(in_maps)
    b2j.install_neuronx_cc_hook()

    part_name = nc.partition_id_tensor.name if nc.partition_id_tensor else None
    in_names, out_names, out_avals, zero_outs = [], [], [], []
    for alloc in nc.m.functions[0].allocations:
        if not isinstance(alloc, mybir.MemoryLocationSet):
            continue
        name = alloc.memorylocations[0].name
        if alloc.kind == "ExternalInput":
            if name != part_name:
                in_names.append(name)
        elif alloc.kind == "ExternalOutput":
            out_names.append(name)
            shp = list(alloc.tensor_shape)
            npdt = mybir.dt.np(alloc.dtype)
            out_avals.append(jax.core.ShapedArray(shp, npdt))
            zero_outs.append(np.zeros(shp, npdt))
    n_params = len(in_names)
    n_outs = len(out_names)
    all_in_names = in_names + out_names
    if part_name is not None:
        all_in_names = all_in_names + [part_name]

    def _body(*args):
        operands = list(args)
        if part_name is not None:
            operands.append(b2j.partition_id_tensor())
        outs = b2j._bass_exec_p.bind(
            *operands, out_avals=tuple(out_avals), in_names=tuple(all_in_names),
            out_names=tuple(out_names), lowering_input_output_aliases=(),
            sim_require_finite=True, sim_require_nnan=True, nc=nc)
        return tuple(outs)

    devices = jax.devices()[:n_cores]
    mesh = Mesh(np.asarray(devices), ("core",))
    in_specs = (PartitionSpec("core"),) * (n_params + n_outs)
    out_specs = (PartitionSpec("core"),) * n_outs
    sharded = jax.jit(shard_map(_body, mesh=mesh, in_specs=in_specs,
                                out_specs=out_specs, check_rep=False),
                      keep_unused=True)
    concat_in = [np.concatenate([np.asarray(m[nm]) for m in in_maps], axis=0)
                 for nm in in_names]
    concat_zeros = [np.zeros((n_cores * z.shape[0], *z.shape[1:]), z.dtype)
                    for z in zero_outs]
    from jax.sharding import NamedSharding
    shard = NamedSharding(mesh, PartitionSpec("core"))
    dev_in = [jax.device_put(a, shard) for a in concat_in]
    dev_zero = [jax.device_put(a, shard) for a in concat_zeros]

    out_arrs = sharded(*dev_in, *dev_zero)           # warmup/compile
    jax.block_until_ready(out_arrs)
    # Per-dispatch wall time is dominated by the axon/PJRT tunnel (~75 ms for
    # a trivial kernel).  Estimate device exec time from the marginal cost of
    # pipelined async dispatches: (T(N2) - T(N1)) / (N2 - N1), which hides
    # the per-call launch overhead (a trivial kernel measures ~0.6 ms here).
    def timed(n):
        t0 = time.perf_counter()
        rs = [sharded(*dev_in, *dev_zero) for _ in range(n)]
        jax.block_until_ready(rs)
        return time.perf_counter() - t0
    n1, n2 = 16, 64
    t1s, t2s = [], []
    for _ in range(max(2, reps)):
        t1s.append(timed(n1))
        t2s.append(timed(n2))
    best = (min(t2s) - min(t1s)) / (n2 - n1)
    out = np.stack([
        np.asarray(out_arrs[0]).reshape(n_cores, *out_avals[0].shape)[c]
        for c in range(n_cores)
    ]).astype(np.float32)
    return out, best * 1e9


if __name__ == "__main__":
    import time
    npz = np.load("/tmp/inputs.npz")
    inputs = {k: npz[k] for k in npz.files}
    t0 = time.time()
    out = kernel(**inputs)
    print(f"kernel done in {time.time()-t0:.1f}s, out shape {out.shape}")
    out2, ns = kernel_timed(**inputs)
    print(f"timed: {ns:.0f} ns  ({ns/1e6:.3f} ms)")

